# revision 1
# baseline (speedup 1.0000x reference)
"""Trainium2 Bass kernel for nn_Attention (dense transformer spatial attention).

Reference computation (per batch b of 4):
  X = x[b] reshaped [256, 4096]                      (4096 = 64*64 pixels)
  QKV = w_qkv @ X -> [384, 4096]; q,k,v = split(QKV) each [128, 4096]
  per head h (4 heads x 32 dims): sim = (q_h*scale)^T k_h   [4096, 4096]
  attn = softmax(sim, axis=-1); out_h = attn @ v_h^T        [4096, 32]
  H = concat_heads -> [128, 4096]; out = w_out @ H + b_out  [256, 4096]

Sharding: 8 cores = (batch b in 0..3) x (query half qh in 0..1).
Each core gets full X_b (for K/V) plus its query-half slice, computes
attention output for its 2048 queries over all 4096 keys, and the final
projection. Gather on host is pure concatenation + transpose (device emits
[i, o] layout).

Device algorithm (per core), designed around engine rooflines (the kernel
is ScalarE-bound: 33.5M softmax exp evaluations per core at 1 elem/lane/
cycle is the hard floor, ~240us; PE/DVE/DMA work hides underneath):
  - Matmuls in float32r (FP22 multiply, 1-pass full-rate on the PE); the
    AV stage uses bf16 operands (the fused f32r weight-load path cannot
    target col-offset PSUM, and mixed f32r/bf16 operands are rejected).
  - sim is computed TRANSPOSED: simT[j, i] = sum_d k[d,j] q[d,i], via
    row-packed K=32 matmuls (one per head, tile_position=(32h,0)), so no
    transposes of the big attention matrix are ever needed. Each step is
    split into two head-pair halves over SEPARATE psum tiles (simA/simB)
    so the next step's matmuls overlap the other half's exp activation.
  - softmax: max-subtraction is skipped (|scale*sim| <~ 20 always, exp is
    safe in f32); scale is folded into the ScalarE exp activation.
  - denominator: V^T is augmented with a ones column then zero-padded to
    M=64, so the AV matmul computes sum_j exp*v AND sum_j exp in one pass.
  - AV: out^T[d_aug, i] = sum_j vTaug[j, d_aug] expT[j, i], accumulated
    over j tiles in PSUM; heads col-packed in pairs at tile_position
    (0,0)/(0,64). Every matmul accumulation group owns whole PSUM banks
    (has_written zeroing is 2KB-region granular, NOT per element).
  - normalization + projection emit outT [i, o] so the per-query softmax
    denominator is applied with plain DVE ops; host transposes back.
  - PSUM budget (8 banks): sim halves 2x[128,1024] = 4 banks, AV pair
    accumulators 2x[128,1024] = 4 banks. Every accumulation group owns
    whole banks; pool-slot WAR is tile-granular, which is why the sim
    halves are separate tiles rather than halves of one tile.
"""

import numpy as np

import concourse.bacc as bacc
import concourse.bass as bass
import concourse.mybir as mybir
import concourse.tile as tile
from concourse.bass_utils import run_bass_kernel_spmd


F32 = mybir.dt.float32
F32R = mybir.dt.float32r
BF16 = mybir.dt.bfloat16

HEADS = 4
DH = 32                      # dim per head
C = 256                      # input channels
NJ = 4096                    # keys per batch (64*64)
NI = 2048                    # queries per core (half of 4096)
JT = 128                     # j tile (partition dim of simT)
NJT = NJ // JT               # 32 j tiles
NT = 512                     # i tile for sim/exp/AV matmuls
CHUNK = 1024                 # i chunk held in AV psum accumulators
NCHUNK = NI // CHUNK         # 2
SCALE = float(DH) ** -0.5
BW = NJ + NI + 3 * 128       # blob256 width


def build_kernel(dbg=False):
    nc = bacc.Bacc("TRN2", debug=False, num_devices=8)

    # blob256 columns: [wqkvT (384) | xq (2048) | x (4096)] - weights and
    # query slice first so the q projection can start while x still streams
    # blob128 columns: [woutA (256) | woutB (256) | bias replicated (256)]
    blob256_d = nc.dram_tensor("blob256", [C, BW], F32R, kind="ExternalInput").ap()
    blob128_d = nc.dram_tensor("blob128", [128, 3 * C], F32R, kind="ExternalInput").ap()
    out_d = nc.dram_tensor("out_t", [NI, C], F32, kind="ExternalOutput").ap()
    # DRAM bounce buffer for partition-broadcasting softmax reciprocals
    # (SBUF->SBUF DMA cannot have a zero partition step on the source).
    rscr_d = nc.dram_tensor("rbscratch", [NCHUNK, 4, CHUNK], F32).ap()
    if dbg:
        dumps = {n: nc.dram_tensor("dump_" + n, s, d, kind="ExternalOutput").ap()
                 for n, s, d in [
                     ("q", [128, NI], F32), ("k", [128, NJ], F32),
                     ("vT", [128, NJT * HEADS * 64], BF16),
                     ("h1", [128, NI], F32), ("h2", [128, NI], F32),
                     ("rb1", [128, CHUNK], F32), ("rb2", [128, CHUNK], F32)]}

    with tile.TileContext(nc) as tc:
        with (
            tc.tile_pool(name="singles", bufs=1) as singles,
            tc.tile_pool(name="expp", bufs=3) as expp,
            tc.tile_pool(name="exp6", bufs=14) as exp6,
            tc.tile_pool(name="outp", bufs=6) as outp,
            tc.tile_pool(name="psim", bufs=1, space="PSUM") as psim,
            tc.tile_pool(name="pav", bufs=2, space="PSUM") as pav,
        ):
            # ---- resident SBUF tensors ----
            blob_sb = singles.tile([128, 2, BW], F32R)    # w|xq|x, 2 c-tiles
            w_sb = blob_sb[:, :, 0:3 * 128]
            xq_sb = blob_sb[:, :, 3 * 128:3 * 128 + NI]
            x_sb = blob_sb[:, :, 3 * 128 + NI:BW]
            b128_sb = singles.tile([128, 3 * C], F32R)
            woutA_sb = b128_sb[:, 0:C]
            woutB_sb = b128_sb[:, C:2 * C]
            bias_sb = b128_sb[:, 2 * C:3 * C]
            q_sb = singles.tile([128, NI], F32R)          # q rows = 4h x 32d
            k_sb = singles.tile([128, NJ], F32R)
            # vT padded to 64 cols: [v dims (32) | ones (1) | zeros (31)]
            # (M=64 keeps the (0,64) col-tiled AV matmul ISA-valid; matmul
            # cost is N-bound so the padding is free)
            # bf16: the fused f32r weight-load path cannot target col-offset
            # PSUM (tile_position (0,64)); bf16 uses the normal LDWEIGHTS path
            vT_sb = singles.tile([128, NJT, HEADS, 64], BF16)
            h1_sb = singles.tile([128, NI], F32R)         # heads 0/1 at rows 0-31/64-95
            h2_sb = singles.tile([128, NI], F32R)         # heads 2/3 at rows 0-31/64-95
            rb1_sb = singles.tile([128, CHUNK], F32)     # 1/denom bcast for h1 rows
            rb2_sb = singles.tile([128, CHUNK], F32)
            dstg_sb = singles.tile([128, 2 * CHUNK], F32)  # epilogue staging

            # single SWDGE queue -> one semaphore for all initial loads
            # (HWDGE round-robins queues and early matmuls then exceed the
            # per-instruction sync-wait slot limit)
            # One contiguous DMA per c-tile: every matmul then transitively
            # depends on exactly ONE DMA (walrus allows only ~2 semaphore
            # waits per instruction, so the wait sets must stay tiny).
            W0 = 3 * 128 + NI          # w + xq prefix
            for ct in range(2):
                nc.sync.dma_start(out=blob_sb[:, ct, 0:W0],
                                  in_=blob256_d[ct * 128:(ct + 1) * 128, 0:W0])
            for ct in range(2):
                for xh in range(4):
                    lo = W0 + xh * (NJ // 4)
                    nc.sync.dma_start(out=blob_sb[:, ct, lo:lo + NJ // 4],
                                      in_=blob256_d[ct * 128:(ct + 1) * 128,
                                                    lo:lo + NJ // 4])
            nc.sync.dma_start(out=b128_sb, in_=blob128_d)

            nc.vector.memset(h1_sb[:, :].bitcast(F32), 0.0)  # unused rows stay 0
            nc.vector.memset(h2_sb[:, :].bitcast(F32), 0.0)
            nc.vector.memset(vT_sb, 0.0)                # zero padding
            nc.vector.memset(vT_sb[:, :, :, DH], 1.0)   # ones column
            nc.vector.memset(rb1_sb, 0.0)
            nc.vector.memset(rb2_sb, 0.0)

            # trigger the ScalarE exp table load (~2.7us) during phase 1
            # instead of at the first real softmax activation
            warm = singles.tile([1, 1], F32)
            nc.vector.memset(warm, 0.0)
            nc.scalar.activation(warm, warm, mybir.ActivationFunctionType.Exp)

            # ---- phase 1: qkv projections ----
            # ordered to match DMA arrival: q needs only w+xq (first DMAs),
            # k-half0/vT(0-15) need x-half0, the rest needs x-half1
            for qg in range(2):
                psq = psim.tile([128, NI // 2], F32, tag="simA" if qg == 0 else "simB")
                for nt in range(2):
                    col = qg * 1024 + nt * 512
                    for ct in range(2):
                        nc.tensor.matmul(
                            psq[:, nt * 512:(nt + 1) * 512],
                            lhsT=w_sb[:, ct, 0:128],
                            rhs=xq_sb[:, ct, col:col + 512],
                            start=(ct == 0), stop=(ct == 1),
                        )
                nc.vector.tensor_copy(q_sb[:, qg * 1024:(qg + 1) * 1024], psq)

            def emit_k_half(half):
                for kg in range(2):
                    psk = psim.tile([128, NI // 2], F32,
                                    tag="simA" if kg == 0 else "simB")
                    for nt in range(2):
                        col = half * NI + kg * 1024 + nt * 512
                        for ct in range(2):
                            nc.tensor.matmul(
                                psk[:, nt * 512:(nt + 1) * 512],
                                lhsT=w_sb[:, ct, 128:256],
                                rhs=x_sb[:, ct, col:col + 512],
                                start=(ct == 0), stop=(ct == 1),
                            )
                    nc.vector.tensor_copy(
                        k_sb[:, half * NI + kg * 1024:half * NI + (kg + 1) * 1024],
                        psk)

            def emit_vt_range(lo, hi):
                # vT[j, vc] = sum_c x[c, j] wv[vc, c], one [128, 128] tile per jt
                for jt in range(lo, hi):
                    psv = pav.tile([128, 128], F32, tag="av")
                    for ct in range(2):
                        nc.tensor.matmul(
                            psv,
                            lhsT=x_sb[:, ct, jt * JT:(jt + 1) * JT],
                            rhs=w_sb[:, ct, 256:384],
                            start=(ct == 0), stop=(ct == 1),
                        )
                    nc.vector.tensor_copy(vT_sb[:, jt, :, 0:DH], psv)

            emit_k_half(0)
            emit_vt_range(0, NJT // 2)
            emit_k_half(1)
            emit_vt_range(NJT // 2, NJT)

            # ---- phase 2: attention main loop ----
            for chunk in range(NCHUNK):
                co = chunk * CHUNK
                avA = pav.tile([128, CHUNK], F32, tag="av")  # heads 0 @0-32, 1 @64-96
                avB = pav.tile([128, CHUNK], F32, tag="av")  # heads 2 @0-32, 3 @64-96
                def emit_av(ex, jt, nt):
                    for h in range(HEADS):
                        av = avA if h < 2 else avB
                        po = 64 * (h % 2)
                        nc.tensor.matmul(
                            av[po:po + 64, nt * NT:(nt + 1) * NT],
                            lhsT=vT_sb[:, jt, h, :],
                            rhs=ex[:, h * NT:(h + 1) * NT],
                            start=(jt == 0), stop=(jt == NJT - 1),
                            tile_position=(0, po),
                            skip_group_check=True,
                        )

                # AV for step s is emitted after sim for step s+1 so the PE
                # unblocks the ScalarE exp (the critical path) first
                # the quad/exp pair is split in two halves over SEPARATE
                # psum tiles (tags simA/simB): the next step's heads-0/1
                # matmuls overlap the current heads-2/3 exp, so the PE never
                # sits on the ScalarE critical path
                pending = None
                for jt in range(NJT):
                    for nt in range(CHUNK // NT):
                        io = co + nt * NT
                        ex = exp6.tile([128, HEADS * NT], BF16, tag="exp")
                        for grp, tag in ((0, "simA"), (1, "simB")):
                            sim = psim.tile([128, 2 * NT], F32, tag=tag)
                            for hi in range(2):
                                h = grp * 2 + hi
                                nc.tensor.matmul(
                                    sim[:, hi * NT:(hi + 1) * NT],
                                    lhsT=k_sb[h * DH:(h + 1) * DH,
                                              jt * JT:(jt + 1) * JT],
                                    rhs=q_sb[h * DH:(h + 1) * DH, io:io + NT],
                                    start=True, stop=True,
                                    tile_position=(h * DH, 0),
                                )
                            nc.scalar.activation(
                                ex[:, grp * 2 * NT:(grp + 1) * 2 * NT], sim,
                                mybir.ActivationFunctionType.Exp, scale=SCALE)
                        if pending is not None:
                            emit_av(*pending)
                        pending = (ex, jt, nt)
                emit_av(*pending)

                # softmax denominators (rows 32 & 96 of the av tiles):
                # bounce the 4 rows through DRAM to pack them into [4, CHUNK]
                # (reciprocal is free-dim bound: one packed call is 4x
                # cheaper than four [1, CHUNK] calls), then reciprocal,
                # bounce back, and partition-broadcast over each head's rows.
                # reciprocal is free-dim bound (8 cyc/elem): repack the
                # 4xCHUNK denominators as [32, CHUNK/8] via the DRAM bounce
                # so the divide runs 8x wider across partitions
                den32 = expp.tile([32, CHUNK // 8], F32, tag="rc")
                rc32 = expp.tile([32, CHUNK // 8], F32, tag="rc")
                dstg = dstg_sb
                for idx, av in enumerate((avA, avB)):
                    for pi, po in enumerate((0, 64)):
                        h4 = idx * 2 + pi
                        cs = idx * CHUNK
                        # DMA cannot read PSUM: stage the row via DVE
                        # (same partition, pair-tiles split by free offset)
                        nc.vector.tensor_copy(dstg[po + DH:po + DH + 1, cs:cs + CHUNK],
                                              av[po + DH:po + DH + 1, :])
                        nc.sync.dma_start(out=rscr_d[chunk, h4, :],
                                          in_=dstg[po + DH:po + DH + 1, cs:cs + CHUNK])
                packed = rscr_d[chunk, :, :].rearrange(
                    "a (b c) -> (a b) c", c=CHUNK // 8)
                nc.sync.dma_start(out=den32, in_=packed)
                nc.vector.reciprocal(out=rc32, in_=den32)
                nc.sync.dma_start(out=packed, in_=rc32)
                for idx, rb in enumerate((rb1_sb, rb2_sb)):
                    for pi, po in enumerate((0, 64)):
                        h4 = idx * 2 + pi
                        nc.sync.dma_start(
                            out=rb[po:po + DH, :],
                            in_=rscr_d[chunk, h4:h4 + 1, :].to_broadcast((DH, CHUNK)),
                        )
                # fused normalize + PSUM->SBUF copy
                for (av, rb, hsb) in ((avA, rb1_sb, h1_sb), (avB, rb2_sb, h2_sb)):
                    for po in (0, 64):
                        nc.vector.tensor_tensor(
                            out=hsb[po:po + DH, co:co + CHUNK],
                            in0=av[po:po + DH, :],
                            in1=rb[po:po + DH, :],
                            op=mybir.AluOpType.mult,
                        )

                # ---- output projection for this chunk ----
                for it in range(CHUNK // 128):
                    io = co + it * 128
                    pj = pav.tile([128, C], F32, tag="av")
                    nc.tensor.matmul(pj, lhsT=h1_sb[:, io:io + 128],
                                     rhs=woutA_sb, start=True, stop=False)
                    nc.tensor.matmul(pj, lhsT=h2_sb[:, io:io + 128],
                                     rhs=woutB_sb, start=False, stop=True)
                    ot = outp.tile([128, C], F32, tag="out")
                    nc.vector.tensor_tensor(out=ot, in0=pj, in1=bias_sb,
                                            op=mybir.AluOpType.add)
                    nc.sync.dma_start(out=out_d[io:io + 128, :], in_=ot)

            if dbg:
                nc.sync.dma_start(out=dumps["q"], in_=q_sb[:, :].bitcast(F32))
                nc.sync.dma_start(out=dumps["k"], in_=k_sb[:, :].bitcast(F32))
                nc.sync.dma_start(out=dumps["vT"],
                                  in_=vT_sb[:, :, :, :].rearrange("p a b c -> p (a b c)"))
                nc.sync.dma_start(out=dumps["h1"], in_=h1_sb[:, :].bitcast(F32))
                nc.sync.dma_start(out=dumps["h2"], in_=h2_sb[:, :].bitcast(F32))
                nc.sync.dma_start(out=dumps["rb1"], in_=rb1_sb)
                nc.sync.dma_start(out=dumps["rb2"], in_=rb2_sb)

    nc.compile()
    return nc


_NC = None


def _get_nc():
    global _NC
    if _NC is None:
        _NC = build_kernel()
    return _NC


def make_in_maps(x, w_qkv, w_out, b_out):
    x = np.ascontiguousarray(np.asarray(x, dtype=np.float32))
    w_qkv = np.asarray(w_qkv, dtype=np.float32)
    w_out = np.asarray(w_out, dtype=np.float32)
    b_out = np.asarray(b_out, dtype=np.float32)

    wqkvT = w_qkv.T                                       # [256, 384]
    woutT = w_out.T                                       # [128 hidden, 256]
    # projection weights permuted to the AV psum partition layout:
    # A: rows 0-31 = head0, rows 64-95 = head1; B: head2, head3; rest zero
    woutA = np.zeros((128, C), np.float32)
    woutB = np.zeros((128, C), np.float32)
    woutA[0:32] = woutT[0:32]
    woutA[64:96] = woutT[32:64]
    woutB[0:32] = woutT[64:96]
    woutB[64:96] = woutT[96:128]
    blob128 = np.ascontiguousarray(
        np.concatenate([woutA, woutB,
                        np.broadcast_to(b_out[None, :], (128, C))], axis=1))

    in_maps = []
    for core in range(8):
        b, qh = divmod(core, 2)
        xb = x[b].reshape(C, NJ)
        xqb = xb[:, qh * NI:(qh + 1) * NI]
        blob256 = np.ascontiguousarray(
            np.concatenate([wqkvT, xqb, xb], axis=1))
        in_maps.append({"blob256": blob256, "blob128": blob128})
    return in_maps


def run_spmd(x, w_qkv, w_out, b_out, **kw):
    nc = _get_nc()
    in_maps = make_in_maps(x, w_qkv, w_out, b_out)
    return run_bass_kernel_spmd(nc, in_maps, core_ids=list(range(8)), **kw)


def assemble(results):
    out = np.empty((4, C, NJ), np.float32)
    for core in range(8):
        b, qh = divmod(core, 2)
        out[b, :, qh * NI:(qh + 1) * NI] = results[core]["out_t"].T
    return out.reshape(4, C, 64, 64)


def kernel(x, w_qkv, w_out, b_out):
    res = run_spmd(x, w_qkv, w_out, b_out)
    return assemble(res.results)



# revision 8
# speedup vs baseline: 1.4638x; 1.4638x over previous
"""Trainium2 Bass kernel for nn_Attention (dense transformer spatial attention).

Reference computation (per batch b of 4):
  X = x[b] reshaped [256, 4096]                      (4096 = 64*64 pixels)
  QKV = w_qkv @ X -> [384, 4096]; q,k,v = split(QKV) each [128, 4096]
  per head h (4 heads x 32 dims): sim = (q_h*scale)^T k_h   [4096, 4096]
  attn = softmax(sim, axis=-1); out_h = attn @ v_h^T        [4096, 32]
  H = concat_heads -> [128, 4096]; out = w_out @ H + b_out  [256, 4096]

Sharding: 8 cores = (batch b in 0..3) x (query half qh in 0..1).
Each core gets full X_b (for K/V) plus its query-half slice, computes
attention output for its 2048 queries over all 4096 keys, and the final
projection.  X columns are permuted per core: [own query half | other half],
so q projections read a contiguous slice and j-order is core-local (softmax
is permutation invariant over keys).

Design notes (engine balance under the TimelineSim cost model):
  - sim is computed TRANSPOSED (simT[j,i]) via K=32 row-packed f32r matmuls
    (tile_position=(32h,0)).  THREE rotating [128,1024] PSUM buffers hold
    sim tiles (6 banks): the serial chain sim->exp->next-sim is ~1.5us deep,
    so two buffers would cap throughput below engine capacity; three make
    the exp engines the limiter.
  - softmax exp is SPLIT between ScalarE (true exp activation, scale folded)
    and the DVE (Schraudolph approximate exp: q is pre-scaled on the host by
    SCALE*128/ln2, so exp bf16 BITS = int16(sim + B) -- one tensor_scalar
    add with fp32->int16 convert writes bf16-bit-pattern output directly).
    The assignment pattern balances both engines at ~150us each.
  - AV is FLIPPED vs the naive layout: stationary = exp tile [128j x 128i],
    moving = vT_aug [128j, 33] (32 v dims + ones column for the softmax
    denominator).  Each matmul costs only N=33 PE cycles, cutting AV PE time
    ~4x.  16 accumulators [128,33] at stride-64 slots share 2 PSUM banks;
    only the first matmul touching each bank uses start=True (has_written
    pending-zero is 2KB-bank granular; later first-touches of other
    accumulators overwrite-on-pending with start=False).
  - The flipped AV emits h as [i, hd]; per 128-query block it is normalized
    (reciprocal of the ones column, free-dim broadcast multiply), PE-
    transposed back to [hd, i] (f32r) and projected with full-width woutT in
    one N=256 f32r matmul.  No DRAM bounces, no partition broadcasts.
  - Epilogue transposes/projections have no PSUM banks of their own: they
    reuse the AV bank-B regions whose accumulators were already consumed by
    the normalize step (matmul start=True pending-zero marks are harmless to
    engine reads; next chunk's first-touch overwrites).  Bank A is never
    touched by the epilogue so the next chunk's AV starts immediately.
  - Phase 1 (QKV projection) evacuations go to ScalarE (idle under the input
    DMA shadow); the second x-half's K/V/vT production is interleaved into
    chunk 0 of the main loop through the same rotating sim buffers.
"""

import math

import numpy as np

import concourse.bacc as bacc
import concourse.bass as bass
import concourse.masks as masks
import concourse.mybir as mybir
import concourse.tile as tile
from concourse.bass_utils import run_bass_kernel_spmd

F32 = mybir.dt.float32
F32R = mybir.dt.float32r
BF16 = mybir.dt.bfloat16
I16 = mybir.dt.int16

HEADS = 4
DH = 32                      # dim per head
C = 256                      # input channels
NJ = 4096                    # keys per batch (64*64)
NI = 2048                    # queries per core (half of 4096)
JT = 128                     # j tile (partition dim of simT)
NJT = NJ // JT               # 32 j tiles
CHUNK = 512                  # i chunk held in AV psum accumulators
NCHUNK = NI // CHUNK         # 4
NIB = CHUNK // 128           # 4 i-blocks per chunk
SCALE = float(DH) ** -0.5
LN2 = math.log(2.0)
# q is pre-scaled by SCALE * 128/ln2 on the host; ScalarE exp then uses
# scale=ln2/128, and the DVE Schraudolph path just adds SCHRAU_B and
# converts to int16 (the bf16 bit pattern of exp).
QPRE = 128.0 / LN2
SCHRAU_C = 0.05              # Schraudolph correction (centers rel err ~+-3%)
SCHRAU_B = 128.0 * (127.0 - SCHRAU_C) + 0.5   # +0.5: f32->i16 truncates

XW = 384 + NJ                # blob256 width: [wq|wk|wv (384) | x perm (4096)]

# (chunk, jt) pairs where the DVE does NOT take an exp group (ScalarE does
# both).  Used to balance ScalarE ~ DVE total busy time and to give the DVE
# air at chunk boundaries (epilogue burst) and phase-1b windows.
DVE_HOLES = {(0, 17), (0, 25)} | {
    (c, jt) for c in (1, 2, 3) for jt in (0, 1, 13, 29)
}


def dve_takes(c, jt):
    """Which exp group (0/1) the DVE handles this step; None = ScalarE both."""
    if (c, jt) in DVE_HOLES:
        return None
    return jt % 2


def build_kernel(dbg=False):
    nc = bacc.Bacc("TRN2", debug=False, num_devices=8)

    blob256_d = nc.dram_tensor("blob256", [C, XW], F32R, kind="ExternalInput").ap()
    blob128_d = nc.dram_tensor("blob128", [128, 2 * C], F32R, kind="ExternalInput").ap()
    out_d = nc.dram_tensor("out_t", [NI, C], F32, kind="ExternalOutput").ap()
    if dbg:
        dumps = {n: nc.dram_tensor("dump_" + n, s, d, kind="ExternalOutput").ap()
                 for n, s, d in [
                     ("q", [128, NI], F32), ("k", [128, NJ], F32),
                     ("v", [128, NJ], BF16),
                     ("vT", [128, NJT * HEADS * 34], BF16),
                     ("rec", [128, NCHUNK * 16], F32),
                     ("ex0", [128, 2048], BF16)]}

    with tile.TileContext(nc) as tc:
        with (
            tc.tile_pool(name="singles", bufs=1) as singles,
            tc.tile_pool(name="expp", bufs=5) as expp,
            tc.tile_pool(name="hp", bufs=4) as hp,
            tc.tile_pool(name="htp", bufs=2) as htp,
            tc.tile_pool(name="otp", bufs=3) as otp,
            tc.tile_pool(name="recp", bufs=2) as recp,
            tc.tile_pool(name="psim", bufs=1, space="PSUM") as psim,
            tc.tile_pool(name="pav", bufs=1, space="PSUM") as pav,
        ):
            # ---- resident SBUF tensors ----
            blob_sb = singles.tile([128, 2, XW], F32R)
            wq_sb = blob_sb[:, :, 0:128]
            wk_sb = blob_sb[:, :, 128:256]
            wv_sb = blob_sb[:, :, 256:384]
            x_sb = blob_sb[:, :, 384:XW]
            b128_sb = singles.tile([128, 2 * C], F32R)
            woutT_sb = b128_sb[:, 0:C]
            bias_sb = b128_sb[:, C:2 * C]
            q_sb = singles.tile([128, NI], F32R)      # rows = 4h x 32d (prescaled)
            k_sb = singles.tile([128, NJ], F32R)
            v_sb = singles.tile([128, NJ], BF16)
            # vT[j, jt, h, 0:32]=v dims, [...,32]=ones, [...,33]=pad
            vT_sb = singles.tile([128, NJT, HEADS, 34], BF16)
            idb_sb = singles.tile([128, 128], BF16)   # identity for bf16 transposes
            idr_sb = singles.tile([128, 128], F32R)   # identity for f32r transposes

            # rotating sim-chain PSUM slot allocator (3 tags x [128,1024])
            SIMTAGS = ("simA", "simB", "simC")
            sim_ctr = [0]

            def sim_tile(shape=(128, 1024), dtype=F32, name="sim"):
                tag = SIMTAGS[sim_ctr[0] % 3]
                sim_ctr[0] += 1
                return psim.tile(list(shape), dtype, tag=tag, name=name)

            # ---- input DMAs (SP engine); x own-query-half first ----
            for ct in range(2):
                nc.sync.dma_start(out=blob_sb[:, ct, 0:384],
                                  in_=blob256_d[ct * 128:(ct + 1) * 128, 0:384])
            nc.sync.dma_start(out=b128_sb, in_=blob128_d)
            for half in range(2):
                for ct in range(2):
                    lo = 384 + half * 2048
                    nc.sync.dma_start(out=blob_sb[:, ct, lo:lo + 2048],
                                      in_=blob256_d[ct * 128:(ct + 1) * 128,
                                                    lo:lo + 2048])

            # identity built once in plain f32 on GpSimd (idle engine), then
            # DVE-converted to the bf16/f32r copies the transposes need
            idf_sb = singles.tile([128, 128], F32)
            masks.make_identity(nc, idf_sb)
            nc.vector.tensor_copy(idb_sb, idf_sb)
            nc.vector.tensor_copy(idr_sb, idf_sb)
            nc.vector.memset(vT_sb[:, :, :, 32], 1.0)

            # trigger the ScalarE exp table load (~2.7us) during phase 1
            warm = singles.tile([1, 1], F32)
            nc.vector.memset(warm, 0.0)
            nc.scalar.activation(warm, warm, mybir.ActivationFunctionType.Exp)

            # ---- phase 1a: q, k/v half 0, vT half 0 ----
            def project(w_slice, x_lo, width, name):
                """[128, width] psum tile = w_slice.T @ x[:, x_lo:x_lo+width]."""
                ps = sim_tile((128, width), F32, name=name)
                for nt in range(width // 512):
                    for ct in range(2):
                        nc.tensor.matmul(
                            ps[:, nt * 512:(nt + 1) * 512],
                            lhsT=w_slice[:, ct, :],
                            rhs=x_sb[:, ct, x_lo + nt * 512:x_lo + (nt + 1) * 512],
                            start=(ct == 0), stop=(ct == 1),
                        )
                return ps

            for qg in range(2):                       # q: own half = x cols 0..2047
                ps = project(wq_sb, qg * 1024, 1024, "ps_q")
                nc.scalar.copy(q_sb[:, qg * 1024:(qg + 1) * 1024], ps)
            for kg in range(2):                       # k half 0
                ps = project(wk_sb, kg * 1024, 1024, "ps_k")
                nc.scalar.copy(k_sb[:, kg * 1024:(kg + 1) * 1024], ps)
            for vg in range(2):                       # v half 0 (bf16)
                ps = project(wv_sb, vg * 1024, 1024, "ps_v")
                nc.scalar.copy(v_sb[:, vg * 1024:(vg + 1) * 1024], ps)

            def emit_vt_quad(g, via_sim):
                """Transpose v j-tiles 4g..4g+3 into vT_sb (PE + one DVE copy)."""
                if via_sim:
                    tp = sim_tile((128, 4, 128), BF16, name="tpq")
                else:
                    tp = pav.tile([128, 4, 128], BF16, tag="av", name="tpq")
                for i4 in range(4):
                    jt = 4 * g + i4
                    nc.tensor.transpose(tp[:, i4, :],
                                        v_sb[:, jt * 128:(jt + 1) * 128], idb_sb)
                g4 = 4 * g
                nc.vector.tensor_copy(
                    vT_sb[:, g4:g4 + 4, :, 0:32],
                    tp[:, :, :].rearrange("p a (h d) -> p a h d", h=HEADS))

            for g in range(4):                        # vT half 0 (through av banks)
                emit_vt_quad(g, via_sim=False)

            # ---- phase 1b pieces, interleaved into chunk 0 of the main loop,
            # rotating through the same sim-chain psum slots.  Evacuations
            # alternate ScalarE (k) / DVE (v) to spread the load.
            def emit_k1_piece(n):
                lo = 2048 + n * 512
                ps = project(wk_sb, lo, 512, "k1p")
                nc.scalar.copy(k_sb[:, lo:lo + 512], ps)

            def emit_v1_piece(n):
                lo = 2048 + n * 512
                ps = project(wv_sb, lo, 512, "v1p")
                nc.vector.tensor_copy(v_sb[:, lo:lo + 512], ps)

            PHASE1B = {
                2: lambda: emit_k1_piece(0), 3: lambda: emit_v1_piece(0),
                4: lambda: emit_k1_piece(1), 5: lambda: emit_v1_piece(1),
                6: lambda: emit_k1_piece(2), 7: lambda: emit_v1_piece(2),
                8: lambda: emit_k1_piece(3), 9: lambda: emit_v1_piece(3),
                10: lambda: emit_vt_quad(4, True), 11: lambda: emit_vt_quad(5, True),
                12: lambda: emit_vt_quad(6, True), 13: lambda: emit_vt_quad(7, True),
            }

            # ---- phase 2: attention main loop ----
            for c in range(NCHUNK):
                i0 = c * CHUNK
                # 16 accumulators [128, 33] at stride-64 slots over 2 banks
                avt = pav.tile([128, 16, 64], F32, tag="av", name="avt")

                def emit_av(ex, jt):
                    for ib in range(NIB):
                        for h in range(HEADS):
                            idx = ib * HEADS + h
                            nc.tensor.matmul(
                                avt[:, idx, 0:33],
                                lhsT=ex[:, h * 512 + ib * 128:h * 512 + (ib + 1) * 128],
                                rhs=vT_sb[:, jt, h, 0:33],
                                start=(jt == 0 and idx % 8 == 0),
                                stop=(jt == NJT - 1),
                                skip_group_check=True,
                            )

                pending = None
                for jt in range(NJT):
                    dve_grp = dve_takes(c, jt)
                    ex = expp.tile([128, HEADS * 512], BF16, tag="exp", name="ex")
                    for grp in range(2):
                        sim = sim_tile()
                        for hi in range(2):
                            h = grp * 2 + hi
                            nc.tensor.matmul(
                                sim[:, hi * 512:(hi + 1) * 512],
                                lhsT=k_sb[h * DH:(h + 1) * DH,
                                          jt * JT:(jt + 1) * JT],
                                rhs=q_sb[h * DH:(h + 1) * DH, i0:i0 + 512],
                                start=True, stop=True,
                                tile_position=(h * DH, 0),
                            )
                        exs = ex[:, grp * 1024:(grp + 1) * 1024]
                        if grp == dve_grp:
                            nc.vector.tensor_scalar(
                                exs.bitcast(I16), sim, SCHRAU_B, None,
                                mybir.AluOpType.add)
                        else:
                            nc.scalar.activation(
                                exs, sim, mybir.ActivationFunctionType.Exp,
                                scale=LN2 / 128.0)
                    if pending is not None:
                        emit_av(*pending)
                    pending = (ex, jt)
                    if c == 0 and jt in PHASE1B:
                        PHASE1B[jt]()
                emit_av(*pending)
                if dbg and c == 0:
                    nc.sync.dma_start(out=dumps["ex0"], in_=pending[0])

                # ---- epilogue: normalize all i-blocks, then transpose +
                # project through the consumed bank-B regions of avt.
                rec = recp.tile([128, 16, 1], F32, tag="rec", name="rec")
                nc.vector.reciprocal(out=rec[:, :, 0], in_=avt[:, :, 32])
                if dbg:
                    nc.sync.dma_start(out=dumps["rec"][:, c * 16:(c + 1) * 16],
                                      in_=rec[:, :, 0])
                hsbs = []
                for ib in range(NIB):
                    hsb = hp.tile([128, HEADS, DH], F32R, tag="h", name="hsb")
                    nc.vector.tensor_tensor(
                        out=hsb,
                        in0=avt[:, ib * HEADS:(ib + 1) * HEADS, 0:32],
                        in1=rec[:, ib * HEADS:(ib + 1) * HEADS, 0:1]
                            .to_broadcast((128, HEADS, DH)),
                        op=mybir.AluOpType.mult,
                    )
                    hsbs.append(hsb.rearrange("p h d -> p (h d)"))
                # two 1KB scratch regions in bank B of avt (accums 8..15),
                # ping-ponged between transpose pairs and projection outputs
                regs = [avt[:, 8:12, :].rearrange("p a b -> p (a b)"),
                        avt[:, 12:16, :].rearrange("p a b -> p (a b)")]
                for pr in range(2):
                    hts = regs[pr].bitcast(F32R)
                    for q2 in range(2):
                        nc.tensor.transpose(hts[:, q2 * 128:(q2 + 1) * 128],
                                            hsbs[pr * 2 + q2], idr_sb)
                    htsb = htp.tile([128, 2, 128], F32R, tag="ht", name="htsb")
                    nc.vector.tensor_copy(htsb, hts.bitcast(F32))
                    for q2 in range(2):
                        ib = pr * 2 + q2
                        io = i0 + ib * 128
                        pj = regs[1 - pr]
                        nc.tensor.matmul(pj, lhsT=htsb[:, q2, :], rhs=woutT_sb,
                                         start=True, stop=True,
                                         skip_group_check=True)
                        ot = otp.tile([128, C], F32, tag="out", name="ot")
                        nc.vector.tensor_tensor(out=ot, in0=pj,
                                                in1=bias_sb.bitcast(F32),
                                                op=mybir.AluOpType.add)
                        nc.sync.dma_start(out=out_d[io:io + 128, :], in_=ot)

            if dbg:
                nc.sync.dma_start(out=dumps["q"], in_=q_sb[:, :].bitcast(F32))
                nc.sync.dma_start(out=dumps["k"], in_=k_sb[:, :].bitcast(F32))
                nc.sync.dma_start(out=dumps["v"], in_=v_sb)
                nc.sync.dma_start(
                    out=dumps["vT"],
                    in_=vT_sb[:, :, :, :].rearrange("p a b c -> p (a b c)"))

    nc.compile()
    return nc


_NC = None


def _get_nc():
    global _NC
    if _NC is None:
        _NC = build_kernel()
    return _NC


def make_in_maps(x, w_qkv, w_out, b_out):
    x = np.ascontiguousarray(np.asarray(x, dtype=np.float32))
    w_qkv = np.asarray(w_qkv, dtype=np.float32)
    w_out = np.asarray(w_out, dtype=np.float32)
    b_out = np.asarray(b_out, dtype=np.float32)

    wqkvT = w_qkv.T.copy()                                # [256, 384]
    wqkvT[:, 0:128] *= SCALE * QPRE                       # fold exp prescale into q
    woutT = w_out.T                                       # [128 hidden, 256]
    blob128 = np.ascontiguousarray(
        np.concatenate([woutT,
                        np.broadcast_to(b_out[None, :], (128, C))], axis=1))

    in_maps = []
    for core in range(8):
        b, qh = divmod(core, 2)
        xb = x[b].reshape(C, NJ)
        # own query half first, then the other half (j-order permuted)
        xp = np.concatenate([xb[:, qh * NI:(qh + 1) * NI],
                             xb[:, (1 - qh) * NI:(2 - qh) * NI]], axis=1)
        blob256 = np.ascontiguousarray(np.concatenate([wqkvT, xp], axis=1))
        in_maps.append({"blob256": blob256, "blob128": blob128})
    return in_maps


def run_spmd(x, w_qkv, w_out, b_out, **kw):
    nc = _get_nc()
    in_maps = make_in_maps(x, w_qkv, w_out, b_out)
    return run_bass_kernel_spmd(nc, in_maps, core_ids=list(range(8)), **kw)


def assemble(results):
    out = np.empty((4, C, NJ), np.float32)
    for core in range(8):
        b, qh = divmod(core, 2)
        out[b, :, qh * NI:(qh + 1) * NI] = results[core]["out_t"].T
    return out.reshape(4, C, 64, 64)


def kernel(x, w_qkv, w_out, b_out):
    res = run_spmd(x, w_qkv, w_out, b_out)
    return assemble(res.results)


# revision 16
# speedup vs baseline: 1.5565x; 1.0633x over previous
"""Trainium2 Bass kernel for nn_Attention (dense transformer spatial attention).

Reference computation (per batch b of 4):
  X = x[b] reshaped [256, 4096]                      (4096 = 64*64 pixels)
  QKV = w_qkv @ X -> [384, 4096]; q,k,v = split(QKV) each [128, 4096]
  per head h (4 heads x 32 dims): sim = (q_h*scale)^T k_h   [4096, 4096]
  attn = softmax(sim, axis=-1); out_h = attn @ v_h^T        [4096, 32]
  H = concat_heads -> [128, 4096]; out = w_out @ H + b_out  [256, 4096]

Sharding: 8 cores = (batch b in 0..3) x (query half qh in 0..1).
Each core gets full X_b (for K/V) plus its query-half slice, computes
attention output for its 2048 queries over all 4096 keys, and the final
projection.  X columns are permuted per core: [own query half | other half],
so q projections read a contiguous slice and j-order is core-local (softmax
is permutation invariant over keys).

Design notes (engine balance under the TimelineSim cost model):
  - sim is computed TRANSPOSED (simT[j,i]) via K=32 row-packed f32r matmuls
    (tile_position=(32h,0)).  THREE rotating [128,1024] PSUM buffers hold
    sim tiles (6 banks): the serial chain sim->exp->next-sim is ~1.5us deep,
    so two buffers would cap throughput below engine capacity; three make
    the exp engines the limiter.
  - softmax exp is SPLIT between ScalarE (true exp activation, scale folded)
    and the DVE (Schraudolph approximate exp: q is pre-scaled on the host by
    SCALE*128/ln2, so exp bf16 BITS = int16(sim + B) -- one tensor_scalar
    add with fp32->int16 convert writes bf16-bit-pattern output directly).
    The assignment pattern balances both engines at ~150us each.
  - AV is FLIPPED vs the naive layout: stationary = exp tile [128j x 128i],
    moving = vT_aug [128j, 33] (32 v dims + ones column for the softmax
    denominator).  Each matmul costs only N=33 PE cycles, cutting AV PE time
    ~4x.  16 accumulators [128,33] at stride-64 slots share 2 PSUM banks;
    only the first matmul touching each bank uses start=True (has_written
    pending-zero is 2KB-bank granular; later first-touches of other
    accumulators overwrite-on-pending with start=False).
  - The flipped AV emits h as [i, hd]; per 128-query block it is normalized
    (reciprocal of the ones column, free-dim broadcast multiply), PE-
    transposed back to [hd, i] (f32r) and projected with full-width woutT in
    one N=256 f32r matmul.  No DRAM bounces, no partition broadcasts.
  - Epilogue transposes/projections have no PSUM banks of their own: they
    reuse the AV bank-B regions whose accumulators were already consumed by
    the normalize step (matmul start=True pending-zero marks are harmless to
    engine reads; next chunk's first-touch overwrites).  Bank A is never
    touched by the epilogue so the next chunk's AV starts immediately.
  - Phase 1 (QKV projection) evacuations go to ScalarE (idle under the input
    DMA shadow); the second x-half's K/V/vT production is interleaved into
    chunk 0 of the main loop through the same rotating sim buffers.
"""

import math

import numpy as np

import concourse.bacc as bacc
import concourse.bass as bass
import concourse.masks as masks
import concourse.mybir as mybir
import concourse.tile as tile
from concourse.bass_utils import run_bass_kernel_spmd

F32 = mybir.dt.float32
F32R = mybir.dt.float32r
BF16 = mybir.dt.bfloat16
I16 = mybir.dt.int16

HEADS = 4
DH = 32                      # dim per head
C = 256                      # input channels
NJ = 4096                    # keys per batch (64*64)
NI = 2048                    # queries per core (half of 4096)
JT = 128                     # j tile (partition dim of simT)
NJT = NJ // JT               # 32 j tiles
CHUNK = 512                  # i chunk held in AV psum accumulators
NCHUNK = NI // CHUNK         # 4
NIB = CHUNK // 128           # 4 i-blocks per chunk
SCALE = float(DH) ** -0.5
LN2 = math.log(2.0)
# q is pre-scaled by SCALE * 128/ln2 on the host; ScalarE exp then uses
# scale=ln2/128, and the DVE Schraudolph path just adds SCHRAU_B and
# converts to int16 (the bf16 bit pattern of exp).
QPRE = 128.0 / LN2
SCHRAU_C = 0.05              # Schraudolph correction (centers rel err ~+-3%)
SCHRAU_B = 128.0 * (127.0 - SCHRAU_C) + 0.5   # +0.5: f32->i16 truncates

XW = 384 + NJ                # blob256 width: [wq|wk|wv (384) | x perm (4096)]

# (chunk, jt) steps where ScalarE additionally takes the first SPLIT_COLS
# columns of the DVE's group-0 tile (a partial hole: smooth rebalancing of
# ScalarE ~ DVE busy time, and air for the DVE's epilogue burst at chunk
# starts without idling it completely).
SPLIT_COLS = 512
SPLIT_STEPS = ({(0, 11), (0, 17), (0, 25)}
               | {(c, jt) for c in (1, 2, 3) for jt in (0, 1, 16)})


def build_kernel(dbg=False):
    nc = bacc.Bacc("TRN2", debug=False, num_devices=8)

    blob256_d = nc.dram_tensor("blob256", [C, XW], F32R, kind="ExternalInput").ap()
    blob128_d = nc.dram_tensor("blob128", [128, 2 * C], F32R, kind="ExternalInput").ap()
    out_d = nc.dram_tensor("out_t", [NI, C], F32, kind="ExternalOutput").ap()
    if dbg:
        dumps = {n: nc.dram_tensor("dump_" + n, s, d, kind="ExternalOutput").ap()
                 for n, s, d in [
                     ("q", [128, NI], F32), ("k", [128, NJ], F32),
                     ("v", [128, NJ], BF16),
                     ("vT", [128, NJT * 128], BF16),
                     ("rec", [128, NCHUNK * 16], F32),
                     ("ex0", [128, 2048], BF16)]}

    with tile.TileContext(nc) as tc:
        with (
            tc.tile_pool(name="singles", bufs=1) as singles,
            tc.tile_pool(name="expp", bufs=6) as expp,
            tc.tile_pool(name="hp", bufs=4) as hp,
            tc.tile_pool(name="htp", bufs=2) as htp,
            tc.tile_pool(name="otp", bufs=3) as otp,
            tc.tile_pool(name="recp", bufs=2) as recp,
            tc.tile_pool(name="psim", bufs=1, space="PSUM") as psim,
            tc.tile_pool(name="pav", bufs=1, space="PSUM") as pav,
        ):
            # ---- resident SBUF tensors ----
            blob_sb = singles.tile([128, 2, XW], F32R)
            wq_sb = blob_sb[:, :, 0:128]
            wk_sb = blob_sb[:, :, 128:256]
            wv_sb = blob_sb[:, :, 256:384]
            x_sb = blob_sb[:, :, 384:XW]
            b128_sb = singles.tile([128, 2 * C], F32R)
            woutT_sb = b128_sb[:, 0:C]
            bias_sb = b128_sb[:, C:2 * C]
            q_sb = singles.tile([128, NI], F32R)      # rows = 4h x 32d (prescaled)
            k_sb = singles.tile([128, NJ], F32R)
            v_sb = singles.tile([128, NJ], BF16)
            # vT[j, jt, hd]: contiguous transposed v tiles (DMA xbar needs a
            # contiguous destination); softmax denominators come from a
            # separate ones column via N=1 matmuls
            vT_sb = singles.tile([128, NJT, 128], BF16)
            onesb_sb = singles.tile([128, 1], BF16)
            idb_sb = singles.tile([128, 128], BF16)   # identity for bf16 transposes
            idr_sb = singles.tile([128, 128], F32R)   # identity for f32r transposes
            ones_sb = singles.tile([1, 128], F32R)    # K=1 bias-broadcast lhsT

            # rotating sim-chain PSUM slot allocator (3 tags x [128,1024])
            SIMTAGS = ("simA", "simB", "simC")
            sim_ctr = [0]

            def sim_tile(shape=(128, 1024), dtype=F32, name="sim"):
                tag = SIMTAGS[sim_ctr[0] % 3]
                sim_ctr[0] += 1
                return psim.tile(list(shape), dtype, tag=tag, name=name)

            # ---- input DMAs (SP engine); x own-query-half first, split in
            # 1024-col pieces so the q/k0/v0 matmuls start as early as possible
            for ct in range(2):
                nc.sync.dma_start(out=blob_sb[:, ct, 0:384],
                                  in_=blob256_d[ct * 128:(ct + 1) * 128, 0:384])
            for piece in range(2):
                for ct in range(2):
                    lo = 384 + piece * 1024
                    nc.sync.dma_start(out=blob_sb[:, ct, lo:lo + 1024],
                                      in_=blob256_d[ct * 128:(ct + 1) * 128,
                                                    lo:lo + 1024])
            for ct in range(2):
                lo = 384 + 2048
                nc.sync.dma_start(out=blob_sb[:, ct, lo:lo + 2048],
                                  in_=blob256_d[ct * 128:(ct + 1) * 128,
                                                lo:lo + 2048])
            nc.sync.dma_start(out=b128_sb, in_=blob128_d)

            # identity built once in plain f32 on GpSimd (idle engine), then
            # DVE-converted to the bf16/f32r copies the transposes need
            idf_sb = singles.tile([128, 128], F32)
            masks.make_identity(nc, idf_sb)
            nc.vector.tensor_copy(idb_sb, idf_sb)
            nc.vector.tensor_copy(idr_sb, idf_sb)
            nc.vector.memset(onesb_sb, 1.0)
            nc.vector.memset(ones_sb.bitcast(F32), 1.0)

            # trigger the ScalarE exp table load (~2.7us) during phase 1
            warm = singles.tile([1, 1], F32)
            nc.vector.memset(warm, 0.0)
            nc.scalar.activation(warm, warm, mybir.ActivationFunctionType.Exp)

            # ---- phase 1a: q, k/v half 0, vT half 0 ----
            def project(w_slice, x_lo, width, name):
                """[128, width] psum tile = w_slice.T @ x[:, x_lo:x_lo+width]."""
                ps = sim_tile((128, width), F32, name=name)
                for nt in range(width // 512):
                    for ct in range(2):
                        nc.tensor.matmul(
                            ps[:, nt * 512:(nt + 1) * 512],
                            lhsT=w_slice[:, ct, :],
                            rhs=x_sb[:, ct, x_lo + nt * 512:x_lo + (nt + 1) * 512],
                            start=(ct == 0), stop=(ct == 1),
                        )
                return ps

            for qg in range(2):                       # q: own half = x cols 0..2047
                ps = project(wq_sb, qg * 1024, 1024, "ps_q")
                nc.scalar.copy(q_sb[:, qg * 1024:(qg + 1) * 1024], ps)
            for kg in range(2):                       # k half 0
                ps = project(wk_sb, kg * 1024, 1024, "ps_k")
                nc.scalar.copy(k_sb[:, kg * 1024:(kg + 1) * 1024], ps)
            for vg in range(2):                       # v half 0 (bf16)
                ps = project(wv_sb, vg * 1024, 1024, "ps_v")
                nc.vector.tensor_copy(v_sb[:, vg * 1024:(vg + 1) * 1024], ps)

            def emit_vt4(g, via_sim):
                """vT tiles 4g..4g+3 via PE transpose + one DVE quad copy.
                (dma_start_transpose is NOT safe here: only the first of a
                burst gets the cross-engine wait on the v copies; the rest
                chain on DMA-queue predecessors and race on hardware.)"""
                if via_sim:
                    tp = sim_tile((128, 4, 128), BF16, name="tpq")
                else:
                    tp = pav.tile([128, 4, 128], BF16, tag="av", name="tpq")
                for i4 in range(4):
                    jt = 4 * g + i4
                    nc.tensor.transpose(tp[:, i4, :],
                                        v_sb[:, jt * 128:(jt + 1) * 128], idb_sb)
                nc.vector.tensor_copy(vT_sb[:, 4 * g:4 * g + 4, :], tp)

            for g in range(4):                        # vT half 0
                emit_vt4(g, via_sim=False)

            # ---- phase 1b pieces, interleaved into chunk 0 of the main loop,
            # rotating through the same sim-chain psum slots.  Evacuations
            # alternate ScalarE (k) / DVE (v) to spread the load.
            def emit_k1_piece(n):
                lo = 2048 + n * 512
                ps = project(wk_sb, lo, 512, "k1p")
                nc.scalar.copy(k_sb[:, lo:lo + 512], ps)

            def emit_v1_piece(n):
                lo = 2048 + n * 512
                ps = project(wv_sb, lo, 512, "v1p")
                nc.vector.tensor_copy(v_sb[:, lo:lo + 512], ps)

            PHASE1B = {
                2: lambda: emit_k1_piece(0), 3: lambda: emit_v1_piece(0),
                4: lambda: emit_k1_piece(1), 5: lambda: emit_v1_piece(1),
                6: lambda: emit_k1_piece(2), 7: lambda: emit_v1_piece(2),
                8: lambda: emit_k1_piece(3), 9: lambda: emit_v1_piece(3),
                10: lambda: emit_vt4(4, True), 11: lambda: emit_vt4(5, True),
                12: lambda: emit_vt4(6, True), 13: lambda: emit_vt4(7, True),
            }

            # ---- phase 2: attention main loop ----
            for c in range(NCHUNK):
                i0 = c * CHUNK
                # 16 accumulators [128, 33] at stride-64 slots over 2 banks
                avt = pav.tile([128, 16, 64], F32, tag="av", name="avt")

                def emit_av(ex, jt):
                    for ib in range(NIB):
                        for h in range(HEADS):
                            idx = ib * HEADS + h
                            exs = ex[:, h * 512 + ib * 128:h * 512 + (ib + 1) * 128]
                            nc.tensor.matmul(
                                avt[:, idx, 0:32],
                                lhsT=exs,
                                rhs=vT_sb[:, jt, h * DH:(h + 1) * DH],
                                start=(jt == 0 and idx % 8 == 0),
                                stop=(jt == NJT - 1),
                                skip_group_check=True,
                            )
                            nc.tensor.matmul(
                                avt[:, idx, 32:33],
                                lhsT=exs,
                                rhs=onesb_sb,
                                start=False, stop=(jt == NJT - 1),
                                skip_group_check=True,
                            )

                # AV emission lags 1 step normally; 3 steps at the start of
                # chunks > 0 so the bank-B WAW (vs the previous epilogue's
                # reads) never parks at the head of the in-order PE queue.
                pending = []
                lag = 3 if c > 0 else 1
                for jt in range(NJT):
                    split = (c, jt) in SPLIT_STEPS
                    ex = expp.tile([128, HEADS * 512], BF16, tag="exp", name="ex")
                    for grp in range(2):
                        sim = sim_tile()
                        for hi in range(2):
                            h = grp * 2 + hi
                            nc.tensor.matmul(
                                sim[:, hi * 512:(hi + 1) * 512],
                                lhsT=k_sb[h * DH:(h + 1) * DH,
                                          jt * JT:(jt + 1) * JT],
                                rhs=q_sb[h * DH:(h + 1) * DH, i0:i0 + 512],
                                start=True, stop=True,
                                tile_position=(h * DH, 0),
                            )
                        exs = ex[:, grp * 1024:(grp + 1) * 1024]
                        if grp == 0:
                            lo = SPLIT_COLS if split else 0
                            if split:
                                nc.scalar.activation(
                                    exs[:, 0:lo], sim[:, 0:lo],
                                    mybir.ActivationFunctionType.Exp,
                                    scale=LN2 / 128.0)
                            nc.vector.tensor_scalar(
                                exs[:, lo:1024].bitcast(I16), sim[:, lo:1024],
                                SCHRAU_B, None, mybir.AluOpType.add)
                        else:
                            nc.scalar.activation(
                                exs, sim, mybir.ActivationFunctionType.Exp,
                                scale=LN2 / 128.0)
                    pending.append((ex, jt))
                    cur_lag = lag if jt < 6 else 1
                    while len(pending) > cur_lag:
                        emit_av(*pending.pop(0))
                    if c == 0 and jt in PHASE1B:
                        PHASE1B[jt]()
                while pending:
                    last_ex = pending[0][0]
                    emit_av(*pending.pop(0))
                if dbg and c == 0:
                    nc.sync.dma_start(out=dumps["ex0"], in_=last_ex)

                # ---- epilogue: normalize all i-blocks, transpose each back
                # to [hd, i] through bank-B regions of avt, then project into
                # the consumed bank-A regions (bias pre-loaded by a K=1
                # ones-row matmul so the final evacuation is a plain copy that
                # either engine can run).
                rec = recp.tile([128, 16, 1], F32, tag="rec", name="rec")
                nc.vector.reciprocal(out=rec[:, :, 0], in_=avt[:, :, 32])
                if dbg:
                    nc.sync.dma_start(out=dumps["rec"][:, c * 16:(c + 1) * 16],
                                      in_=rec[:, :, 0])
                hsbs = []
                for ib in range(NIB):
                    hsb = hp.tile([128, HEADS, DH], F32R, tag="h", name="hsb")
                    nc.vector.tensor_tensor(
                        out=hsb,
                        in0=avt[:, ib * HEADS:(ib + 1) * HEADS, 0:32],
                        in1=rec[:, ib * HEADS:(ib + 1) * HEADS, 0:1]
                            .to_broadcast((128, HEADS, DH)),
                        op=mybir.AluOpType.mult,
                    )
                    hsbs.append(hsb.rearrange("p h d -> p (h d)"))
                # all 4 transposes into bank B (accums 8..15, 512B each)
                tps = avt[:, 8:16, :].rearrange("p a b -> p (a b)").bitcast(F32R)
                for ib in range(NIB):
                    nc.tensor.transpose(tps[:, ib * 128:(ib + 1) * 128],
                                        hsbs[ib], idr_sb)
                htsb = htp.tile([128, 4, 128], F32R, tag="ht", name="htsb")
                nc.scalar.copy(htsb[:, 0:2, :],
                               tps[:, 0:256].bitcast(F32))
                nc.vector.tensor_copy(htsb[:, 2:4, :],
                                      tps[:, 256:512].bitcast(F32))
                # projections ping-pong through bank A (accums 0..7)
                pjregs = [avt[:, 0:4, :].rearrange("p a b -> p (a b)"),
                          avt[:, 4:8, :].rearrange("p a b -> p (a b)")]
                for ib in range(NIB):
                    io = i0 + ib * 128
                    pj = pjregs[ib % 2]
                    nc.tensor.matmul(pj, lhsT=ones_sb,
                                     rhs=bias_sb[0:1, :], start=True,
                                     stop=False, skip_group_check=True)
                    nc.tensor.matmul(pj, lhsT=htsb[:, ib, :], rhs=woutT_sb,
                                     start=False, stop=True,
                                     skip_group_check=True)
                    ot = otp.tile([128, C], F32, tag="out", name="ot")
                    if ib % 2 == 0:
                        nc.vector.tensor_copy(ot, pj)
                    else:
                        nc.scalar.copy(ot, pj)
                    nc.sync.dma_start(out=out_d[io:io + 128, :], in_=ot)

            if dbg:
                nc.sync.dma_start(out=dumps["q"], in_=q_sb[:, :].bitcast(F32))
                nc.sync.dma_start(out=dumps["k"], in_=k_sb[:, :].bitcast(F32))
                nc.sync.dma_start(out=dumps["v"], in_=v_sb)
                nc.sync.dma_start(
                    out=dumps["vT"],
                    in_=vT_sb[:, :, :].rearrange("p a b -> p (a b)"))

    nc.compile()
    return nc


_NC = None


def _get_nc():
    global _NC
    if _NC is None:
        _NC = build_kernel()
    return _NC


def make_in_maps(x, w_qkv, w_out, b_out):
    x = np.ascontiguousarray(np.asarray(x, dtype=np.float32))
    w_qkv = np.asarray(w_qkv, dtype=np.float32)
    w_out = np.asarray(w_out, dtype=np.float32)
    b_out = np.asarray(b_out, dtype=np.float32)

    wqkvT = w_qkv.T.copy()                                # [256, 384]
    wqkvT[:, 0:128] *= SCALE * QPRE                       # fold exp prescale into q
    woutT = w_out.T                                       # [128 hidden, 256]
    blob128 = np.ascontiguousarray(
        np.concatenate([woutT,
                        np.broadcast_to(b_out[None, :], (128, C))], axis=1))

    in_maps = []
    for core in range(8):
        b, qh = divmod(core, 2)
        xb = x[b].reshape(C, NJ)
        # own query half first, then the other half (j-order permuted)
        xp = np.concatenate([xb[:, qh * NI:(qh + 1) * NI],
                             xb[:, (1 - qh) * NI:(2 - qh) * NI]], axis=1)
        blob256 = np.ascontiguousarray(np.concatenate([wqkvT, xp], axis=1))
        in_maps.append({"blob256": blob256, "blob128": blob128})
    return in_maps


def run_spmd(x, w_qkv, w_out, b_out, **kw):
    nc = _get_nc()
    in_maps = make_in_maps(x, w_qkv, w_out, b_out)
    return run_bass_kernel_spmd(nc, in_maps, core_ids=list(range(8)), **kw)


def assemble(results):
    out = np.empty((4, C, NJ), np.float32)
    for core in range(8):
        b, qh = divmod(core, 2)
        out[b, :, qh * NI:(qh + 1) * NI] = results[core]["out_t"].T
    return out.reshape(4, C, 64, 64)


def kernel(x, w_qkv, w_out, b_out):
    res = run_spmd(x, w_qkv, w_out, b_out)
    return assemble(res.results)
